# revision 1
# baseline (speedup 1.0000x reference)
"""Trainium2 Bass kernel for nn_CrossAttentionBlock.

Reference computation (per batch b):
  Q = wq @ x1   [32, 4096]     (x1 = feat1[b] reshaped [256, HW])
  K = wk @ x2   [32, 4096]
  V = wv @ x2   [256, 4096]
  A = softmax_j(Q^T K / sqrt(32))      [4096, 4096]
  out[c, i] = sum_j V[c, j] A[i, j]    [256, 4096]

Sharding: 8 cores = 4 batches x 2 query-halves (2048 queries each).
Each core gets x1 = feat1[b][:, half] and the full x2 = feat2[b].

Device layout: everything is computed with keys (j) in the partition
dimension so no transposes are ever needed:
  S^T[j, i] tiles via matmul(lhsT=K[:, jtile], rhs=Q[:, ichunk])
      (Q/K stored zero-padded to 128 partitions: K=128 matmuls are ~2x
       faster than K=32 on TRN2)
  E = exp(S^T * scale) via ScalarE, batched 2 jtiles per ACTIVATE
  numerator  += matmul(lhsT=V'^T[jtile, cslice], rhs=E)    (accum over jtiles)
      where V' = [V ; ones] and the c-slices are 80/80/97 rows, so the
      denominator comes out as output row 96 of the third slice - no
      separate denominator matmul.
  out = numerator * broadcast(1/denom)   (broadcast via K=1 ones matmul)
V^T is produced directly by matmul(lhsT=x2[:, jtile], rhs=wv^T).
Projections and the reciprocal path run in float32r (full PE rate at
tf32-ish precision); the hot S^T/AV matmuls use bf16 operands (fast
weight load) accumulating into fp32 PSUM. Measured ~187us/core steady
state, rel err ~2e-3 vs the fp32 reference.
"""

import numpy as np

import concourse.bass as bass
import concourse.tile as tile
from concourse import bacc, mybir
from concourse._compat import axon_active

f32 = mybir.dt.float32
f32r = mybir.dt.float32r
bf16 = mybir.dt.bfloat16

B, C, H, W = 4, 256, 64, 64
HW = H * W            # 4096
D = 32                # q/k channels
NCORES = 8
IHALF = HW // 2       # 2048 queries per core
NI = 512              # query chunk (psum bank)
NJT = HW // 128       # 32 key tiles
NIC = IHALF // NI     # 4 query chunks
GRP = 2               # jtiles per exp batch (2 x 512 = one 2-bank S^T tile)
SCALE = 1.0 / np.sqrt(np.float32(D))

_CACHE = {}
Exp = mybir.ActivationFunctionType.Exp
AluAdd = mybir.AluOpType.add


def _build(has_bv: bool, loop_n: int = 1):
    nc = bacc.Bacc("TRN2", target_bir_lowering=False, debug=False, num_devices=NCORES)

    x1 = nc.dram_tensor("x1", [C, IHALF], f32r, kind="ExternalInput").ap()
    x2 = nc.dram_tensor("x2", [C, HW], f32r, kind="ExternalInput").ap()
    wqT = nc.dram_tensor("wqT", [C, 128], f32r, kind="ExternalInput").ap()
    wkT = nc.dram_tensor("wkT", [C, 128], f32r, kind="ExternalInput").ap()
    wvT = nc.dram_tensor("wvT", [C, C], f32r, kind="ExternalInput").ap()
    bq = nc.dram_tensor("bq", [128, 1], f32, kind="ExternalInput").ap()
    bk = nc.dram_tensor("bk", [128, 1], f32, kind="ExternalInput").ap()
    bv = nc.dram_tensor("bv", [1, C], f32r, kind="ExternalInput").ap()
    out = nc.dram_tensor("out", [C, IHALF], f32, kind="ExternalOutput").ap()

    with tile.TileContext(nc) as tc:
        with tc.tile_pool(name="persist", bufs=1) as per, \
             tc.tile_pool(name="xpool", bufs=1) as xp, \
             tc.tile_pool(name="qk", bufs=1) as qkp, \
             tc.tile_pool(name="vt", bufs=NJT) as vtp, \
             tc.tile_pool(name="ps", bufs=1, space="PSUM") as ps, \
             tc.tile_pool(name="epool", bufs=3) as epool, \
             tc.tile_pool(name="opool", bufs=4) as opool:
            # constants (loaded once, outside any timing loop)
            bq_sb = per.tile([128, 1], f32, tag="bq", name="bq")
            bk_sb = per.tile([128, 1], f32, tag="bk", name="bk")
            bv_sb = per.tile([1, C], f32r, tag="bv", name="bv")
            nc.sync.dma_start(bq_sb[:], bq[:])
            nc.sync.dma_start(bk_sb[:], bk[:])
            if has_bv:
                nc.sync.dma_start(bv_sb[:], bv[:])
            ones_f = per.tile([128, 2], f32, tag="ones_f", name="ones_f")
            nc.vector.memset(ones_f[:], 1.0)
            ones_r = per.tile([128, 2], f32r, tag="ones_r", name="ones_r")
            nc.vector.tensor_copy(ones_r[:], ones_f[:])
            ones_col = ones_r[:, 0:1]        # [128, 1] lhsT for denominator
            ones_rowf = per.tile([1, 128], f32, tag="ones_rowf", name="ones_rowf")
            nc.vector.memset(ones_rowf[:], 1.0)
            ones_row = per.tile([1, 128], f32r, tag="ones_row", name="ones_row")
            nc.vector.tensor_copy(ones_row[:], ones_rowf[:])
            wq_sb = [per.tile([128, 128], f32r, tag=f"wq_{k}", name=f"wq_{k}") for k in range(2)]
            wk_sb = [per.tile([128, 128], f32r, tag=f"wk_{k}", name=f"wk_{k}") for k in range(2)]
            wv_sb = [per.tile([128, C], f32r, tag=f"wv_{k}", name=f"wv_{k}") for k in range(2)]
            for k in range(2):
                sl = slice(128 * k, 128 * (k + 1))
                nc.sync.dma_start(wq_sb[k][:], wqT[sl, :])
                nc.sync.dma_start(wk_sb[k][:], wkT[sl, :])
                nc.sync.dma_start(wv_sb[k][:], wvT[sl, :])

            def body():
                # --- load activations ---
                x1_sb = [xp.tile([128, IHALF], f32r, tag=f"x1_{k}", name=f"x1_{k}")
                         for k in range(2)]
                x2_sb = [xp.tile([128, HW], f32r, tag=f"x2_{k}", name=f"x2_{k}")
                         for k in range(2)]
                for k in range(2):
                    sl = slice(128 * k, 128 * (k + 1))
                    nc.sync.dma_start(x2_sb[k][:], x2[sl, :])
                    nc.sync.dma_start(x1_sb[k][:], x1[sl, :])

                # Q/K stored zero-padded to 128 partitions (rows 32..127 = 0)
                q_sb = qkp.tile([128, IHALF], bf16, tag="q", name="q")
                k_sb = qkp.tile([128, HW], bf16, tag="k", name="k")
                vt_sb = []

                # V^T tiles: [128 j, 256 c] = x2[:, jt]^T @ wv^T
                # (psum slots borrowed from the av0/av1 banks, free during proj)
                for t in range(NJT):
                    pv = ps.tile([128, C], f32, tag=f"av{t % 2}", bufs=1,
                                 name="pv", padded_shape=[128, NI])
                    js = slice(128 * t, 128 * (t + 1))
                    nc.tensor.matmul(pv[:], x2_sb[0][:, js], wv_sb[0][:],
                                     start=True, stop=False)
                    nc.tensor.matmul(pv[:], x2_sb[1][:, js], wv_sb[1][:],
                                     start=False, stop=not has_bv)
                    if has_bv:
                        nc.tensor.matmul(pv[:], ones_row[:], bv_sb[:],
                                         start=False, stop=True)
                    vt = vtp.tile([128, C + 1], bf16, tag="vt", name="vt")
                    nc.vector.tensor_copy(vt[:, 0:C], pv[:])
                    nc.vector.tensor_copy(vt[:, C:C + 1], ones_f[:, 0:1])
                    vt_sb.append(vt)

                # K then Q: [32, *] in chunks of 512, bias-added on DVE
                for ic in range(HW // NI):
                    pk = ps.tile([128, NI], f32, tag=f"av{ic % 2}", bufs=1, name="pk")
                    cs = slice(NI * ic, NI * (ic + 1))
                    nc.tensor.matmul(pk[:], wk_sb[0][:], x2_sb[0][:, cs],
                                     start=True, stop=False)
                    nc.tensor.matmul(pk[:], wk_sb[1][:], x2_sb[1][:, cs],
                                     start=False, stop=True)
                    nc.vector.tensor_scalar(k_sb[:, cs], pk[:], bk_sb[:, 0:1], None, AluAdd)
                for ic in range(NIC):
                    pq = ps.tile([128, NI], f32, tag=f"av{ic % 2}", bufs=1, name="pq")
                    cs = slice(NI * ic, NI * (ic + 1))
                    nc.tensor.matmul(pq[:], wq_sb[0][:], x1_sb[0][:, cs],
                                     start=True, stop=False)
                    nc.tensor.matmul(pq[:], wq_sb[1][:], x1_sb[1][:, cs],
                                     start=False, stop=True)
                    nc.vector.tensor_scalar(q_sb[:, cs], pq[:], bq_sb[:, 0:1], None, AluAdd)

                # --- attention ---
                NG = NJT // GRP     # 8 groups of 4 jtiles
                CS = [(0, 80), (80, 160), (160, 257)]   # last: 96 channels + denom row @ p96
                for ic in range(NIC):
                    qs = q_sb[:, NI * ic:NI * (ic + 1)]
                    av = [ps.tile([b - a, NI], f32, tag=f"av{ct}", bufs=1, name=f"av{ct}")
                          for ct, (a, b) in enumerate(CS)]

                    def emit_av(e, g):
                        for k in range(GRP):
                            t = g * GRP + k
                            er = e[:, NI * k:NI * (k + 1)]
                            first, last = t == 0, t == NJT - 1
                            for ct, (a, b) in enumerate(CS):
                                nc.tensor.matmul(av[ct][:],
                                                 vt_sb[t][:, a:b],
                                                 er, start=first, stop=last)

                    prevs = []
                    for g in range(NG):
                        st = ps.tile([128, GRP * NI], f32, tag="st", bufs=2, name="st")
                        for k in range(GRP):
                            t = g * GRP + k
                            nc.tensor.matmul(st[:, NI * k:NI * (k + 1)],
                                             k_sb[:, 128 * t:128 * (t + 1)], qs,
                                             start=True, stop=True)
                        if len(prevs) == 2:
                            emit_av(*prevs.pop(0))
                        e = epool.tile([128, GRP * NI], bf16, tag="e", name="e")
                        nc.scalar.activation(e[:], st[:], Exp, scale=float(SCALE))
                        prevs.append((e, g))
                    for p in prevs:
                        emit_av(*p)

                    # normalize: out = av * broadcast(1/den); denom = last row of av[2]
                    recip = opool.tile([1, NI], f32, tag="recip", name="recip")
                    nc.vector.reciprocal(recip[:], av[2][96:97, :])
                    recip_r = opool.tile([1, NI], f32r, tag="recip_r", name="recip_r")
                    nc.vector.tensor_copy(recip_r[:], recip[:])
                    bc = ps.tile([128, NI], f32, tag="bc", bufs=1, name="bc")
                    nc.tensor.matmul(bc[:], ones_row[:], recip_r[:],
                                     start=True, stop=True)
                    bc_sb = opool.tile([128, NI], f32, tag="bc_sb", name="bc_sb")
                    nc.vector.tensor_copy(bc_sb[:], bc[:])
                    for ct, (a, b) in enumerate(CS):
                        rows = min(b, C) - a
                        o = opool.tile([rows, NI], f32, tag="o", name="o")
                        nc.vector.tensor_mul(o[:], av[ct][0:rows, :], bc_sb[0:rows, :])
                        nc.sync.dma_start(out[a:a + rows, NI * ic:NI * (ic + 1)], o[:])

            if loop_n == 1:
                body()
            else:
                with tc.For_i(0, loop_n, 1, hint_engines=(mybir.EngineType.PE,
                                                          mybir.EngineType.Activation)):
                    body()

    nc.compile()
    return nc


class _Runner:
    """Compiled 8-core PJRT executable, reusable across calls (no donation)."""

    def __init__(self, nc):
        import jax
        from jax.sharding import Mesh, PartitionSpec
        from jax.experimental.shard_map import shard_map
        from concourse import bass2jax

        bass2jax.install_neuronx_cc_hook()
        self.jax = jax
        self.nc = nc
        partition_name = nc.partition_id_tensor.name if nc.partition_id_tensor else None
        in_names, out_names, out_avals, zero_outs = [], [], [], []
        for alloc in nc.m.functions[0].allocations:
            if not isinstance(alloc, mybir.MemoryLocationSet):
                continue
            name = alloc.memorylocations[0].name
            if alloc.kind == "ExternalInput":
                if name != partition_name:
                    in_names.append(name)
            elif alloc.kind == "ExternalOutput":
                out_names.append(name)
                shape = tuple(alloc.tensor_shape)
                dtype = mybir.dt.np(alloc.dtype)
                out_avals.append(jax.core.ShapedArray(shape, dtype))
                zero_outs.append(np.zeros(shape, dtype))
        self.in_names, self.out_names, self.out_avals = in_names, out_names, out_avals
        all_names = list(in_names) + out_names
        if partition_name is not None:
            all_names.append(partition_name)

        def _body(*args):
            operands = list(args)
            if partition_name is not None:
                operands.append(bass2jax.partition_id_tensor())
            outs = bass2jax._bass_exec_p.bind(
                *operands,
                out_avals=tuple(out_avals),
                in_names=tuple(all_names),
                out_names=tuple(out_names),
                lowering_input_output_aliases=(),
                sim_require_finite=True,
                sim_require_nnan=True,
                nc=nc,
            )
            return tuple(outs)

        devices = jax.devices()[:NCORES]
        mesh = Mesh(np.asarray(devices), ("core",))
        n_params, n_outs = len(in_names), len(out_names)
        in_specs = (PartitionSpec("core"),) * (n_params + n_outs)
        out_specs = (PartitionSpec("core"),) * n_outs
        self.fn = jax.jit(
            shard_map(_body, mesh=mesh, in_specs=in_specs, out_specs=out_specs,
                      check_rep=False),
            keep_unused=True,
        )
        self.zero_args = [
            jax.device_put(np.zeros((NCORES * z.shape[0], *z.shape[1:]), z.dtype))
            for z in zero_outs
        ]

    def prep(self, in_maps):
        per_core = [[np.asarray(m[name]) for name in self.in_names] for m in in_maps]
        concat = [np.concatenate([per_core[c][i] for c in range(NCORES)], axis=0)
                  for i in range(len(self.in_names))]
        return [self.jax.device_put(a) for a in concat] + self.zero_args

    def run(self, args):
        outs = self.fn(*args)
        self.jax.block_until_ready(outs)
        return outs

    def unshard(self, outs):
        return [
            {name: np.asarray(outs[i]).reshape(NCORES, *self.out_avals[i].shape)[c]
             for i, name in enumerate(self.out_names)}
            for c in range(NCORES)
        ]

    def __call__(self, in_maps):
        return self.unshard(self.run(self.prep(in_maps)))


def _get_runner(has_bv: bool, loop_n: int = 1):
    key = (has_bv, loop_n)
    if key not in _CACHE:
        nc = _build(has_bv, loop_n)
        if axon_active():
            _CACHE[key] = _Runner(nc)
        else:
            from concourse.bass_utils import run_bass_kernel_spmd

            def native(in_maps, _nc=nc):
                res = run_bass_kernel_spmd(_nc, in_maps, core_ids=list(range(NCORES)))
                return res.results
            _CACHE[key] = native
    return _CACHE[key]


def _make_in_maps(inputs):
    feat1 = np.ascontiguousarray(np.asarray(inputs["feat1"], dtype=np.float32))
    feat2 = np.ascontiguousarray(np.asarray(inputs["feat2"], dtype=np.float32))
    wq = np.asarray(inputs["wq"], dtype=np.float32)
    wk = np.asarray(inputs["wk"], dtype=np.float32)
    wv = np.asarray(inputs["wv"], dtype=np.float32)
    bq = np.zeros((128, 1), np.float32)
    bq[:D, 0] = np.asarray(inputs["bq"], dtype=np.float32).ravel()
    bk = np.zeros((128, 1), np.float32)
    bk[:D, 0] = np.asarray(inputs["bk"], dtype=np.float32).ravel()
    bv = np.asarray(inputs["bv"], dtype=np.float32).reshape(1, C)
    wqT = np.zeros((C, 128), np.float32)
    wqT[:, :D] = wq.T
    wkT = np.zeros((C, 128), np.float32)
    wkT[:, :D] = wk.T
    wvT = np.ascontiguousarray(wv.T)
    f1 = feat1.reshape(B, C, HW)
    f2 = feat2.reshape(B, C, HW)
    in_maps = []
    for core in range(NCORES):
        b, half = divmod(core, 2)
        in_maps.append({
            "x1": np.ascontiguousarray(f1[b][:, IHALF * half:IHALF * (half + 1)]),
            "x2": f2[b],
            "wqT": wqT, "wkT": wkT, "wvT": wvT,
            "bq": bq, "bk": bk, "bv": bv,
        })
    return in_maps, bool(np.any(bv))


def kernel(**inputs) -> np.ndarray:
    in_maps, has_bv = _make_in_maps(inputs)
    runner = _get_runner(has_bv)
    results = runner(in_maps)
    out = np.empty((B, C, HW), dtype=np.float32)
    for core in range(NCORES):
        b, half = divmod(core, 2)
        out[b][:, IHALF * half:IHALF * (half + 1)] = results[core]["out"]
    return out.reshape(B, C, H, W)



# revision 5
# speedup vs baseline: 1.7835x; 1.7835x over previous
"""Trainium2 Bass kernel for nn_CrossAttentionBlock.

Reference computation (per batch b):
  Q = wq @ x1   [32, 4096]     (x1 = feat1[b] reshaped [256, HW])
  K = wk @ x2   [32, 4096]
  V = wv @ x2   [256, 4096]
  A = softmax_j(Q^T K / sqrt(32))      [4096, 4096]
  out[c, i] = sum_j V[c, j] A[i, j]    [256, 4096]

Sharding: 8 cores = 4 batches x 2 query-halves (2048 queries each).
Each core gets x1 = feat1[b][:, half] and the full x2 = feat2[b].

Device layout: keys (j) live in the partition dimension end to end:
  S^T[j, i] tiles via matmul(lhsT=K[:, jtile], rhs=Q[:, ichunk])  (bf16)
  E = exp(S^T * scale) on ACT, written as fp8e4 in paired layout
      [128, 2 jtiles, 512 i]  (one ACTIVATE per 2-jtile PSUM group)
  AV is FLIPPED vs the naive orientation: E is the stationary operand,
  V'^T = [V ; ones] the moving one, so each DoubleRow fp8 matmul emits
  out^T[i-subtile(128), 257] and contracts 256 j at 0.5 cyc/row:
      matmul(po[s], lhsT=E[:, :, 128s:128s+128], rhs=vt2[g][128, 2, 257])
  accumulated over the 16 jtile pairs. Channel 256 (the ones column of
  V') lands the softmax denominator in the same psum tile, so the
  normalize is a per-partition DVE reciprocal + tensor_scalar multiply.
  Output is written transposed ([i, c]); the host flips it back.
V^T is produced directly by matmul(lhsT=x2[:, jtile], rhs=wv^T) and
converted to fp8e4 on DVE. Projections run in float32r.
"""

import numpy as np

import concourse.bass as bass
import concourse.tile as tile
from concourse import bacc, mybir
from concourse._compat import axon_active

f32 = mybir.dt.float32
f32r = mybir.dt.float32r
bf16 = mybir.dt.bfloat16
fp8 = mybir.dt.float8e4

B, C, H, W = 4, 256, 64, 64
HW = H * W            # 4096
D = 32                # q/k channels
NCORES = 8
IHALF = HW // 2       # 2048 queries per core
NI = 512              # query chunk (psum bank)
NJT = HW // 128       # 32 key tiles
NIC = IHALF // NI     # 4 query chunks
NG = NJT // 2         # 16 jtile pairs
SCALE = 1.0 / np.sqrt(np.float32(D))

_CACHE = {}
Exp = mybir.ActivationFunctionType.Exp
AluAdd = mybir.AluOpType.add
AluMult = mybir.AluOpType.mult
DR = mybir.MatmulPerfMode.DoubleRow


def _build(has_bv: bool, loop_n: int = 1):
    nc = bacc.Bacc("TRN2", target_bir_lowering=False, debug=False, num_devices=NCORES)

    x1 = nc.dram_tensor("x1", [C, IHALF], f32r, kind="ExternalInput").ap()
    x2 = nc.dram_tensor("x2", [C, HW], f32r, kind="ExternalInput").ap()
    wqT = nc.dram_tensor("wqT", [C, 128], f32r, kind="ExternalInput").ap()
    wkT = nc.dram_tensor("wkT", [C, 128], f32r, kind="ExternalInput").ap()
    wvT = nc.dram_tensor("wvT", [C, C], f32r, kind="ExternalInput").ap()
    bq = nc.dram_tensor("bq", [128, 1], f32, kind="ExternalInput").ap()
    bk = nc.dram_tensor("bk", [128, 1], f32, kind="ExternalInput").ap()
    bv = nc.dram_tensor("bv", [1, C], f32r, kind="ExternalInput").ap()
    # out is [i, c] (transposed); host flips back
    out = nc.dram_tensor("out", [IHALF, C], f32, kind="ExternalOutput").ap()

    with tile.TileContext(nc) as tc:
        with tc.tile_pool(name="persist", bufs=1) as per, \
             tc.tile_pool(name="xpool", bufs=1) as xp, \
             tc.tile_pool(name="qk", bufs=1) as qkp, \
             tc.tile_pool(name="ps", bufs=1, space="PSUM") as ps, \
             tc.tile_pool(name="epool", bufs=3) as epool, \
             tc.tile_pool(name="opool", bufs=4) as opool:
            # constants (loaded once, outside any timing loop)
            bq_sb = per.tile([128, 1], f32, tag="bq", name="bq")
            bk_sb = per.tile([128, 1], f32, tag="bk", name="bk")
            bv_sb = per.tile([1, C], f32r, tag="bv", name="bv")
            nc.sync.dma_start(bq_sb[:], bq[:])
            nc.sync.dma_start(bk_sb[:], bk[:])
            if has_bv:
                nc.sync.dma_start(bv_sb[:], bv[:])
            # bias -2 keeps exp() in fp8e4 range (max logit ~6.7 -> e^4.7
            # = 110 < 240); cancels in the softmax ratio
            ebias = per.tile([128, 1], f32, tag="ebias", name="ebias")
            nc.vector.memset(ebias[:], -2.0)
            ones_rowf = per.tile([1, 128], f32, tag="ones_rowf", name="ones_rowf")
            nc.vector.memset(ones_rowf[:], 1.0)
            ones_row = per.tile([1, 128], f32r, tag="ones_row", name="ones_row")
            nc.vector.tensor_copy(ones_row[:], ones_rowf[:])
            wq_sb = [per.tile([128, 128], f32r, tag=f"wq_{k}", name=f"wq_{k}") for k in range(2)]
            wk_sb = [per.tile([128, 128], f32r, tag=f"wk_{k}", name=f"wk_{k}") for k in range(2)]
            wv_sb = [per.tile([128, C], f32r, tag=f"wv_{k}", name=f"wv_{k}") for k in range(2)]
            for k in range(2):
                sl = slice(128 * k, 128 * (k + 1))
                nc.sync.dma_start(wq_sb[k][:], wqT[sl, :])
                nc.sync.dma_start(wk_sb[k][:], wkT[sl, :])
                nc.sync.dma_start(wv_sb[k][:], wvT[sl, :])
            # V'^T tiles, paired for DoubleRow: [128 j, 2 jtiles, 257 c].
            # Column 256 is the all-ones denominator channel, set once here
            # and never rewritten inside the loop.
            vt2 = [per.tile([128, 2, C + 1], fp8, tag=f"vt{g}", name=f"vt{g}")
                   for g in range(NG)]
            for g in range(NG):
                nc.vector.memset(vt2[g][:, :, C:C + 1], 1.0)

            def body():
                # --- load activations ---
                x1_sb = [xp.tile([128, IHALF], f32r, tag=f"x1_{k}", name=f"x1_{k}")
                         for k in range(2)]
                x2_sb = [xp.tile([128, HW], f32r, tag=f"x2_{k}", name=f"x2_{k}")
                         for k in range(2)]
                for k in range(2):
                    sl = slice(128 * k, 128 * (k + 1))
                    nc.sync.dma_start(x2_sb[k][:], x2[sl, :])
                    nc.sync.dma_start(x1_sb[k][:], x1[sl, :])

                # Q/K stored zero-padded to 128 partitions (rows 32..127 = 0)
                q_sb = qkp.tile([128, IHALF], bf16, tag="q", name="q")
                k_sb = qkp.tile([128, HW], bf16, tag="k", name="k")

                # V^T tiles: [128 j, 256 c] = x2[:, jt]^T @ wv^T, then fp8
                for t in range(NJT):
                    pv = ps.tile([128, C], f32, tag=f"po{t % 2}", bufs=1,
                                 name="pv", padded_shape=[128, NI])
                    js = slice(128 * t, 128 * (t + 1))
                    nc.tensor.matmul(pv[:], x2_sb[0][:, js], wv_sb[0][:],
                                     start=True, stop=False)
                    nc.tensor.matmul(pv[:], x2_sb[1][:, js], wv_sb[1][:],
                                     start=False, stop=not has_bv)
                    if has_bv:
                        nc.tensor.matmul(pv[:], ones_row[:], bv_sb[:],
                                         start=False, stop=True)
                    nc.vector.tensor_copy(vt2[t // 2][:, t % 2, 0:C], pv[:])

                # K then Q: [32, *] in chunks of 512, bias-added on DVE
                for ic in range(HW // NI):
                    pk = ps.tile([128, NI], f32, tag=f"po{2 + ic % 2}", bufs=1, name="pk")
                    cs = slice(NI * ic, NI * (ic + 1))
                    nc.tensor.matmul(pk[:], wk_sb[0][:], x2_sb[0][:, cs],
                                     start=True, stop=False)
                    nc.tensor.matmul(pk[:], wk_sb[1][:], x2_sb[1][:, cs],
                                     start=False, stop=True)
                    nc.vector.tensor_scalar(k_sb[:, cs], pk[:], bk_sb[:, 0:1], None, AluAdd)
                for ic in range(NIC):
                    pq = ps.tile([128, NI], f32, tag=f"po{2 + ic % 2}", bufs=1, name="pq")
                    cs = slice(NI * ic, NI * (ic + 1))
                    nc.tensor.matmul(pq[:], wq_sb[0][:], x1_sb[0][:, cs],
                                     start=True, stop=False)
                    nc.tensor.matmul(pq[:], wq_sb[1][:], x1_sb[1][:, cs],
                                     start=False, stop=True)
                    nc.vector.tensor_scalar(q_sb[:, cs], pq[:], bq_sb[:, 0:1], None, AluAdd)

                # --- attention ---
                for ic in range(NIC):
                    qs = q_sb[:, NI * ic:NI * (ic + 1)]
                    # out^T accumulators, one per 128-query subtile
                    po = [ps.tile([128, C + 1], f32, tag=f"po{s}", bufs=1,
                                  name=f"po{s}", padded_shape=[128, NI])
                          for s in range(4)]

                    def emit_av(e, g):
                        first, last = g == 0, g == NG - 1
                        for s in range(4):
                            nc.tensor.matmul(po[s][:],
                                             e[:, :, 128 * s:128 * (s + 1)],
                                             vt2[g][:], start=first, stop=last,
                                             perf_mode=DR)

                    prevs = []
                    for g in range(NG):
                        st = ps.tile([128, 2, NI], f32, tag="st", bufs=2, name="st")
                        for k in range(2):
                            t = 2 * g + k
                            nc.tensor.matmul(st[:, k, :],
                                             k_sb[:, 128 * t:128 * (t + 1)], qs,
                                             start=True, stop=True)
                        if len(prevs) == 2:
                            emit_av(*prevs.pop(0))
                        e = epool.tile([128, 2, NI], fp8, tag="e", name="e")
                        nc.scalar.activation(e[:], st[:], Exp, scale=float(SCALE),
                                             bias=ebias[:, 0:1])
                        prevs.append((e, g))
                    for p in prevs:
                        emit_av(*p)

                    # normalize: out^T[i, c] = po[i, c] / po[i, 256]
                    for s in range(4):
                        recip = opool.tile([128, 1], f32, tag="recip", name="recip")
                        nc.vector.reciprocal(recip[:], po[s][:, C:C + 1])
                        o = opool.tile([128, C], f32, tag="o", name="o")
                        nc.vector.tensor_scalar(o[:], po[s][:, 0:C], recip[:, 0:1],
                                                None, AluMult)
                        r0 = NI * ic + 128 * s
                        nc.sync.dma_start(out[r0:r0 + 128, :], o[:])

            if loop_n == 1:
                body()
            else:
                with tc.For_i(0, loop_n, 1, hint_engines=(mybir.EngineType.PE,
                                                          mybir.EngineType.Activation)):
                    body()

    nc.compile()
    return nc


class _Runner:
    """Compiled 8-core PJRT executable, reusable across calls (no donation)."""

    def __init__(self, nc):
        import jax
        from jax.sharding import Mesh, PartitionSpec
        from jax.experimental.shard_map import shard_map
        from concourse import bass2jax

        bass2jax.install_neuronx_cc_hook()
        self.jax = jax
        self.nc = nc
        partition_name = nc.partition_id_tensor.name if nc.partition_id_tensor else None
        in_names, out_names, out_avals, zero_outs = [], [], [], []
        for alloc in nc.m.functions[0].allocations:
            if not isinstance(alloc, mybir.MemoryLocationSet):
                continue
            name = alloc.memorylocations[0].name
            if alloc.kind == "ExternalInput":
                if name != partition_name:
                    in_names.append(name)
            elif alloc.kind == "ExternalOutput":
                out_names.append(name)
                shape = tuple(alloc.tensor_shape)
                dtype = mybir.dt.np(alloc.dtype)
                out_avals.append(jax.core.ShapedArray(shape, dtype))
                zero_outs.append(np.zeros(shape, dtype))
        self.in_names, self.out_names, self.out_avals = in_names, out_names, out_avals
        all_names = list(in_names) + out_names
        if partition_name is not None:
            all_names.append(partition_name)

        def _body(*args):
            operands = list(args)
            if partition_name is not None:
                operands.append(bass2jax.partition_id_tensor())
            outs = bass2jax._bass_exec_p.bind(
                *operands,
                out_avals=tuple(out_avals),
                in_names=tuple(all_names),
                out_names=tuple(out_names),
                lowering_input_output_aliases=(),
                sim_require_finite=True,
                sim_require_nnan=True,
                nc=nc,
            )
            return tuple(outs)

        devices = jax.devices()[:NCORES]
        mesh = Mesh(np.asarray(devices), ("core",))
        n_params, n_outs = len(in_names), len(out_names)
        in_specs = (PartitionSpec("core"),) * (n_params + n_outs)
        out_specs = (PartitionSpec("core"),) * n_outs
        self.fn = jax.jit(
            shard_map(_body, mesh=mesh, in_specs=in_specs, out_specs=out_specs,
                      check_rep=False),
            keep_unused=True,
        )
        self.zero_args = [
            jax.device_put(np.zeros((NCORES * z.shape[0], *z.shape[1:]), z.dtype))
            for z in zero_outs
        ]

    def prep(self, in_maps):
        per_core = [[np.asarray(m[name]) for name in self.in_names] for m in in_maps]
        concat = [np.concatenate([per_core[c][i] for c in range(NCORES)], axis=0)
                  for i in range(len(self.in_names))]
        return [self.jax.device_put(a) for a in concat] + self.zero_args

    def run(self, args):
        outs = self.fn(*args)
        self.jax.block_until_ready(outs)
        return outs

    def unshard(self, outs):
        return [
            {name: np.asarray(outs[i]).reshape(NCORES, *self.out_avals[i].shape)[c]
             for i, name in enumerate(self.out_names)}
            for c in range(NCORES)
        ]

    def __call__(self, in_maps):
        return self.unshard(self.run(self.prep(in_maps)))


def _get_runner(has_bv: bool, loop_n: int = 1):
    key = (has_bv, loop_n)
    if key not in _CACHE:
        nc = _build(has_bv, loop_n)
        if axon_active():
            _CACHE[key] = _Runner(nc)
        else:
            from concourse.bass_utils import run_bass_kernel_spmd

            def native(in_maps, _nc=nc):
                res = run_bass_kernel_spmd(_nc, in_maps, core_ids=list(range(NCORES)))
                return res.results
            _CACHE[key] = native
    return _CACHE[key]


def _make_in_maps(inputs):
    feat1 = np.ascontiguousarray(np.asarray(inputs["feat1"], dtype=np.float32))
    feat2 = np.ascontiguousarray(np.asarray(inputs["feat2"], dtype=np.float32))
    wq = np.asarray(inputs["wq"], dtype=np.float32)
    wk = np.asarray(inputs["wk"], dtype=np.float32)
    wv = np.asarray(inputs["wv"], dtype=np.float32)
    bq = np.zeros((128, 1), np.float32)
    bq[:D, 0] = np.asarray(inputs["bq"], dtype=np.float32).ravel()
    bk = np.zeros((128, 1), np.float32)
    bk[:D, 0] = np.asarray(inputs["bk"], dtype=np.float32).ravel()
    bv = np.asarray(inputs["bv"], dtype=np.float32).reshape(1, C)
    wqT = np.zeros((C, 128), np.float32)
    wqT[:, :D] = wq.T
    wkT = np.zeros((C, 128), np.float32)
    wkT[:, :D] = wk.T
    wvT = np.ascontiguousarray(wv.T)
    f1 = feat1.reshape(B, C, HW)
    f2 = feat2.reshape(B, C, HW)
    in_maps = []
    for core in range(NCORES):
        b, half = divmod(core, 2)
        in_maps.append({
            "x1": np.ascontiguousarray(f1[b][:, IHALF * half:IHALF * (half + 1)]),
            "x2": f2[b],
            "wqT": wqT, "wkT": wkT, "wvT": wvT,
            "bq": bq, "bk": bk, "bv": bv,
        })
    return in_maps, bool(np.any(bv))


def kernel(**inputs) -> np.ndarray:
    in_maps, has_bv = _make_in_maps(inputs)
    runner = _get_runner(has_bv)
    results = runner(in_maps)
    out = np.empty((B, C, HW), dtype=np.float32)
    for core in range(NCORES):
        b, half = divmod(core, 2)
        out[b][:, IHALF * half:IHALF * (half + 1)] = results[core]["out"].T
    return out.reshape(B, C, H, W)


# revision 8
# speedup vs baseline: 2.2753x; 1.2757x over previous
"""Trainium2 Bass kernel for nn_CrossAttentionBlock.

Reference computation (per batch b):
  Q = wq @ x1   [32, 4096]     (x1 = feat1[b] reshaped [256, HW])
  K = wk @ x2   [32, 4096]
  V = wv @ x2   [256, 4096]
  A = softmax_j(Q^T K / sqrt(32))      [4096, 4096]
  out[c, i] = sum_j V[c, j] A[i, j]    [256, 4096]

Sharding: 8 cores = 4 batches x 2 query-halves (2048 queries each).
Each core gets x1 = feat1[b][:, half] and the full x2 = feat2[b].

Device layout: keys (j) live in the partition dimension end to end.
Per 256-query chunk (8 chunks per core):
  S^T[j, i] via matmul(lhsT=K[:, jtile], rhs=Q[:, ichunk]), 4 jtiles per
      2-bank PSUM tile [128, 4, 256]  (bf16 operands)
  E = exp(S^T/sqrt(32) - 2) on ACT -> fp8e4, one ACTIVATE per st tile.
      The -2 bias keeps exp() under the fp8e4 max (~240) and cancels in
      the softmax ratio. ACT is the bottleneck engine (~66us/iter);
      everything else is shaped to hide under it.
  AV is flipped: E is stationary, V'^T = [V ; ones] moving, DoubleRow
      fp8 contracts 256 keys per matmul at 0.5 cyc/row:
        matmul(po[s], lhsT=E[:, pair, 128s:128s+128], rhs=vt2[g])
      po[s] = out^T[i-subtile(128), 257] accumulates over 16 pairs in a
      single PSUM bank; channel 256 is the softmax denominator, so
      normalize = per-partition DVE reciprocal + multiply. Output is
      written transposed ([i, c]); the host flips it back.
V^T is matmul(lhsT=x2[:, jtile], rhs=wv^T) in f32r, fp8-converted on DVE.

Software pipelining: the For_i timing loop unrolls 2 iterations with
parity-swapped buffer sets; iteration n's projections (x-DMA, V^T, K, Q)
are emitted interleaved into iteration n-1's attention chunks so the PE
does them in its slack while ACT stays saturated. PSUM: st(2x2 banks) +
po(2) + proj(2) = 8 banks.
"""

import numpy as np

import concourse.bass as bass
import concourse.tile as tile
from concourse import bacc, mybir
from concourse._compat import axon_active

f32 = mybir.dt.float32
f32r = mybir.dt.float32r
bf16 = mybir.dt.bfloat16
fp8 = mybir.dt.float8e4

B, C, H, W = 4, 256, 64, 64
HW = H * W            # 4096
D = 32                # q/k channels
NCORES = 8
IHALF = HW // 2       # 2048 queries per core
NI = 256              # query chunk
NJT = HW // 128       # 32 key tiles
NIC = IHALF // NI     # 8 query chunks
NG = NJT // 2         # 16 jtile pairs (DoubleRow groups)
NST = NJT // 4        # 8 st tiles per query chunk (4 jtiles each)
SCALE = 1.0 / np.sqrt(np.float32(D))

_CACHE = {}
Exp = mybir.ActivationFunctionType.Exp
AluAdd = mybir.AluOpType.add
AluMult = mybir.AluOpType.mult
DR = mybir.MatmulPerfMode.DoubleRow


def _build(has_bv: bool, loop_n: int = 1, unrolled: bool = False):
    assert loop_n == 1 or loop_n % 2 == 0, "loop_n must be 1 or even"
    nc = bacc.Bacc("TRN2", target_bir_lowering=False, debug=False, num_devices=NCORES)

    x1 = nc.dram_tensor("x1", [C, IHALF], f32r, kind="ExternalInput").ap()
    x2 = nc.dram_tensor("x2", [C, HW], f32r, kind="ExternalInput").ap()
    wqT = nc.dram_tensor("wqT", [C, 128], f32r, kind="ExternalInput").ap()
    wkT = nc.dram_tensor("wkT", [C, 128], f32r, kind="ExternalInput").ap()
    wvT = nc.dram_tensor("wvT", [C, C], f32r, kind="ExternalInput").ap()
    bq = nc.dram_tensor("bq", [128, 1], f32, kind="ExternalInput").ap()
    bk = nc.dram_tensor("bk", [128, 1], f32, kind="ExternalInput").ap()
    bv = nc.dram_tensor("bv", [1, C], f32r, kind="ExternalInput").ap()
    # out is [i, c] (transposed); host flips back
    out = nc.dram_tensor("out", [IHALF, C], f32, kind="ExternalOutput").ap()

    NPAR = 1 if loop_n == 1 else 2   # buffer parity sets

    with tile.TileContext(nc) as tc:
        with tc.tile_pool(name="persist", bufs=1) as per, \
             tc.tile_pool(name="ps", bufs=1, space="PSUM") as ps, \
             tc.tile_pool(name="epool", bufs=3) as epool, \
             tc.tile_pool(name="opool", bufs=4) as opool:
            # constants (loaded once, outside any timing loop)
            bq_sb = per.tile([128, 1], f32, tag="bq", name="bq")
            bk_sb = per.tile([128, 1], f32, tag="bk", name="bk")
            bv_sb = per.tile([1, C], f32r, tag="bv", name="bv")
            nc.sync.dma_start(bq_sb[:], bq[:])
            nc.sync.dma_start(bk_sb[:], bk[:])
            if has_bv:
                nc.sync.dma_start(bv_sb[:], bv[:])
            # bias -2 keeps exp() in fp8e4 range (max logit ~6.7 -> e^4.7
            # = 110 < 240); cancels in the softmax ratio
            ebias = per.tile([128, 1], f32, tag="ebias", name="ebias")
            nc.vector.memset(ebias[:], -2.0)
            ones_rowf = per.tile([1, 128], f32, tag="ones_rowf", name="ones_rowf")
            nc.vector.memset(ones_rowf[:], 1.0)
            ones_row = per.tile([1, 128], f32r, tag="ones_row", name="ones_row")
            nc.vector.tensor_copy(ones_row[:], ones_rowf[:])
            wq_sb = [per.tile([128, 128], f32r, tag=f"wq_{k}", name=f"wq_{k}") for k in range(2)]
            wk_sb = [per.tile([128, 128], f32r, tag=f"wk_{k}", name=f"wk_{k}") for k in range(2)]
            wv_sb = [per.tile([128, C], f32r, tag=f"wv_{k}", name=f"wv_{k}") for k in range(2)]
            for k in range(2):
                sl = slice(128 * k, 128 * (k + 1))
                nc.sync.dma_start(wq_sb[k][:], wqT[sl, :])
                nc.sync.dma_start(wk_sb[k][:], wkT[sl, :])
                nc.sync.dma_start(wv_sb[k][:], wvT[sl, :])

            # per-parity buffer sets
            x1_sb = [[per.tile([128, IHALF], f32r, tag=f"x1_{p}_{k}", name=f"x1_{p}_{k}")
                      for k in range(2)] for p in range(NPAR)]
            x2_sb = [[per.tile([128, HW], f32r, tag=f"x2_{p}_{k}", name=f"x2_{p}_{k}")
                      for k in range(2)] for p in range(NPAR)]
            q_sb = [per.tile([128, IHALF], bf16, tag=f"q_{p}", name=f"q_{p}")
                    for p in range(NPAR)]
            k_sb = [per.tile([128, HW], bf16, tag=f"k_{p}", name=f"k_{p}")
                    for p in range(NPAR)]
            # V'^T pairs for DoubleRow: [128 j, 2 jtiles, 257 c]; col 256 is
            # the all-ones denominator channel, set once, never rewritten.
            vt2 = [[per.tile([128, 2, C + 1], fp8, tag=f"vt{p}_{g}", name=f"vt{p}_{g}")
                    for g in range(NG)] for p in range(NPAR)]
            for p in range(NPAR):
                for g in range(NG):
                    nc.vector.memset(vt2[p][g][:, :, C:C + 1], 1.0)

            def load_x(p):
                for k in range(2):
                    sl = slice(128 * k, 128 * (k + 1))
                    nc.sync.dma_start(x2_sb[p][k][:], x2[sl, :])
                    nc.sync.dma_start(x1_sb[p][k][:], x1[sl, :])

            def proj_v(p, t0, t1):
                for t in range(t0, t1):
                    pv = ps.tile([128, C], f32, tag=f"pj{t % 2}", bufs=1,
                                 name="pv", padded_shape=[128, 512])
                    js = slice(128 * t, 128 * (t + 1))
                    nc.tensor.matmul(pv[:], x2_sb[p][0][:, js], wv_sb[0][:],
                                     start=True, stop=False)
                    nc.tensor.matmul(pv[:], x2_sb[p][1][:, js], wv_sb[1][:],
                                     start=False, stop=not has_bv)
                    if has_bv:
                        nc.tensor.matmul(pv[:], ones_row[:], bv_sb[:],
                                         start=False, stop=True)
                    nc.vector.tensor_copy(vt2[p][t // 2][:, t % 2, 0:C], pv[:])

            def proj_k(p, c0, c1):
                for c in range(c0, c1):
                    pk = ps.tile([128, 512], f32, tag=f"pj{c % 2}", bufs=1, name="pk")
                    cs = slice(512 * c, 512 * (c + 1))
                    nc.tensor.matmul(pk[:], wk_sb[0][:], x2_sb[p][0][:, cs],
                                     start=True, stop=False)
                    nc.tensor.matmul(pk[:], wk_sb[1][:], x2_sb[p][1][:, cs],
                                     start=False, stop=True)
                    nc.vector.tensor_scalar(k_sb[p][:, cs], pk[:], bk_sb[:, 0:1],
                                            None, AluAdd)

            def proj_q(p, c0, c1):
                for c in range(c0, c1):
                    pq = ps.tile([128, 512], f32, tag=f"pj{c % 2}", bufs=1, name="pq")
                    cs = slice(512 * c, 512 * (c + 1))
                    nc.tensor.matmul(pq[:], wq_sb[0][:], x1_sb[p][0][:, cs],
                                     start=True, stop=False)
                    nc.tensor.matmul(pq[:], wq_sb[1][:], x1_sb[p][1][:, cs],
                                     start=False, stop=True)
                    nc.vector.tensor_scalar(q_sb[p][:, cs], pq[:], bq_sb[:, 0:1],
                                            None, AluAdd)

            def attention_chunk(p, ic):
                qs = q_sb[p][:, NI * ic:NI * (ic + 1)]
                po = [ps.tile([128, C + 1], f32, tag=f"po{s}", bufs=1,
                              name=f"po{s}", padded_shape=[128, 512])
                      for s in range(2)]

                def emit_av(e, gb):
                    for u2 in range(2):
                        g = 2 * gb + u2
                        first = gb == 0 and u2 == 0
                        last = gb == NST - 1 and u2 == 1
                        for s in range(2):
                            nc.tensor.matmul(po[s][:],
                                             e[:, 2 * u2:2 * u2 + 2,
                                               128 * s:128 * (s + 1)],
                                             vt2[p][g][:], start=first, stop=last,
                                             perf_mode=DR)

                prevs = []
                for gb in range(NST):
                    st = ps.tile([128, 4, NI], f32, tag="st", bufs=2, name="st")
                    for u in range(4):
                        t = 4 * gb + u
                        nc.tensor.matmul(st[:, u, :],
                                         k_sb[p][:, 128 * t:128 * (t + 1)], qs,
                                         start=True, stop=True)
                    if len(prevs) == 2:
                        emit_av(*prevs.pop(0))
                    e = epool.tile([128, 4, NI], fp8, tag="e", name="e")
                    nc.scalar.activation(e[:], st[:], Exp, scale=float(SCALE),
                                         bias=ebias[:, 0:1])
                    prevs.append((e, gb))
                for pr in prevs:
                    emit_av(*pr)

                # normalize: out^T[i, c] = po[i, c] / po[i, 256]
                for s in range(2):
                    recip = opool.tile([128, 1], f32, tag="recip", name="recip")
                    nc.vector.reciprocal(recip[:], po[s][:, C:C + 1])
                    o = opool.tile([128, C], f32, tag="o", name="o")
                    nc.vector.tensor_scalar(o[:], po[s][:, 0:C], recip[:, 0:1],
                                            None, AluMult)
                    r0 = NI * ic + 128 * s
                    nc.sync.dma_start(out[r0:r0 + 128, :], o[:])

            def half(cur, nxt, do_proj):
                if do_proj:
                    load_x(nxt)
                for ic in range(NIC):
                    attention_chunk(cur, ic)
                    if do_proj:
                        # spread next iteration's projections into PE slack;
                        # V^T starts at ic>=3 so the x2 DMA has a head start
                        if 3 <= ic <= 6:
                            proj_v(nxt, 8 * (ic - 3), 8 * (ic - 2))
                        elif ic == 7:
                            proj_k(nxt, 0, 8)
                if do_proj:
                    proj_q(nxt, 0, 4)

            # prologue: fill parity-0 buffers
            load_x(0)
            proj_v(0, 0, NJT)
            proj_k(0, 0, 8)
            proj_q(0, 0, 4)

            if loop_n == 1:
                half(0, 0, False)
            elif unrolled:
                # python-unrolled variant (For_i hangs the no-exec cost sim)
                for _ in range(loop_n // 2):
                    half(0, 1, True)
                    half(1, 0, True)
            else:
                with tc.For_i(0, loop_n // 2, 1,
                              hint_engines=(mybir.EngineType.PE,
                                            mybir.EngineType.Activation)):
                    half(0, 1, True)
                    half(1, 0, True)

    nc.compile()
    return nc


class _Runner:
    """Compiled 8-core PJRT executable, reusable across calls (no donation)."""

    def __init__(self, nc):
        import jax
        from jax.sharding import Mesh, PartitionSpec
        from jax.experimental.shard_map import shard_map
        from concourse import bass2jax

        bass2jax.install_neuronx_cc_hook()
        self.jax = jax
        self.nc = nc
        partition_name = nc.partition_id_tensor.name if nc.partition_id_tensor else None
        in_names, out_names, out_avals, zero_outs = [], [], [], []
        for alloc in nc.m.functions[0].allocations:
            if not isinstance(alloc, mybir.MemoryLocationSet):
                continue
            name = alloc.memorylocations[0].name
            if alloc.kind == "ExternalInput":
                if name != partition_name:
                    in_names.append(name)
            elif alloc.kind == "ExternalOutput":
                out_names.append(name)
                shape = tuple(alloc.tensor_shape)
                dtype = mybir.dt.np(alloc.dtype)
                out_avals.append(jax.core.ShapedArray(shape, dtype))
                zero_outs.append(np.zeros(shape, dtype))
        self.in_names, self.out_names, self.out_avals = in_names, out_names, out_avals
        all_names = list(in_names) + out_names
        if partition_name is not None:
            all_names.append(partition_name)

        def _body(*args):
            operands = list(args)
            if partition_name is not None:
                operands.append(bass2jax.partition_id_tensor())
            outs = bass2jax._bass_exec_p.bind(
                *operands,
                out_avals=tuple(out_avals),
                in_names=tuple(all_names),
                out_names=tuple(out_names),
                lowering_input_output_aliases=(),
                sim_require_finite=True,
                sim_require_nnan=True,
                nc=nc,
            )
            return tuple(outs)

        devices = jax.devices()[:NCORES]
        mesh = Mesh(np.asarray(devices), ("core",))
        n_params, n_outs = len(in_names), len(out_names)
        in_specs = (PartitionSpec("core"),) * (n_params + n_outs)
        out_specs = (PartitionSpec("core"),) * n_outs
        self.fn = jax.jit(
            shard_map(_body, mesh=mesh, in_specs=in_specs, out_specs=out_specs,
                      check_rep=False),
            keep_unused=True,
        )
        self.zero_args = [
            jax.device_put(np.zeros((NCORES * z.shape[0], *z.shape[1:]), z.dtype))
            for z in zero_outs
        ]

    def prep(self, in_maps):
        per_core = [[np.asarray(m[name]) for name in self.in_names] for m in in_maps]
        concat = [np.concatenate([per_core[c][i] for c in range(NCORES)], axis=0)
                  for i in range(len(self.in_names))]
        return [self.jax.device_put(a) for a in concat] + self.zero_args

    def run(self, args):
        outs = self.fn(*args)
        self.jax.block_until_ready(outs)
        return outs

    def unshard(self, outs):
        return [
            {name: np.asarray(outs[i]).reshape(NCORES, *self.out_avals[i].shape)[c]
             for i, name in enumerate(self.out_names)}
            for c in range(NCORES)
        ]

    def __call__(self, in_maps):
        return self.unshard(self.run(self.prep(in_maps)))


def _get_runner(has_bv: bool, loop_n: int = 1):
    key = (has_bv, loop_n)
    if key not in _CACHE:
        nc = _build(has_bv, loop_n)
        if axon_active():
            _CACHE[key] = _Runner(nc)
        else:
            from concourse.bass_utils import run_bass_kernel_spmd

            def native(in_maps, _nc=nc):
                res = run_bass_kernel_spmd(_nc, in_maps, core_ids=list(range(NCORES)))
                return res.results
            _CACHE[key] = native
    return _CACHE[key]


def _make_in_maps(inputs):
    feat1 = np.ascontiguousarray(np.asarray(inputs["feat1"], dtype=np.float32))
    feat2 = np.ascontiguousarray(np.asarray(inputs["feat2"], dtype=np.float32))
    wq = np.asarray(inputs["wq"], dtype=np.float32)
    wk = np.asarray(inputs["wk"], dtype=np.float32)
    wv = np.asarray(inputs["wv"], dtype=np.float32)
    bq = np.zeros((128, 1), np.float32)
    bq[:D, 0] = np.asarray(inputs["bq"], dtype=np.float32).ravel()
    bk = np.zeros((128, 1), np.float32)
    bk[:D, 0] = np.asarray(inputs["bk"], dtype=np.float32).ravel()
    bv = np.asarray(inputs["bv"], dtype=np.float32).reshape(1, C)
    wqT = np.zeros((C, 128), np.float32)
    wqT[:, :D] = wq.T
    wkT = np.zeros((C, 128), np.float32)
    wkT[:, :D] = wk.T
    wvT = np.ascontiguousarray(wv.T)
    f1 = feat1.reshape(B, C, HW)
    f2 = feat2.reshape(B, C, HW)
    in_maps = []
    for core in range(NCORES):
        b, half = divmod(core, 2)
        in_maps.append({
            "x1": np.ascontiguousarray(f1[b][:, IHALF * half:IHALF * (half + 1)]),
            "x2": f2[b],
            "wqT": wqT, "wkT": wkT, "wvT": wvT,
            "bq": bq, "bk": bk, "bv": bv,
        })
    return in_maps, bool(np.any(bv))


def kernel(**inputs) -> np.ndarray:
    in_maps, has_bv = _make_in_maps(inputs)
    runner = _get_runner(has_bv)
    results = runner(in_maps)
    out = np.empty((B, C, HW), dtype=np.float32)
    for core in range(NCORES):
        b, half = divmod(core, 2)
        out[b][:, IHALF * half:IHALF * (half + 1)] = results[core]["out"].T
    return out.reshape(B, C, H, W)
